# revision 13
# baseline (speedup 1.0000x reference)
"""CapsNet Trainium2 kernel: 8-core SPMD Bass/Tile implementation.

Strategy:
  Phase 1 (contraction-parallel): the dominant op is
     dct_emb = relu(norm(log|DCT|) @ W_emb.T + b_emb),  [512,102400]x[102400,768]
  Each core owns a 12800-wide slice of the 102400 contraction dim.
  The global mean/std normalization is affine, so it folds into the
  matmul epilogue:  (G - mu*s_w)/sigma + b = G/sigma + beta,
  with W pre-divided by sigma on host and beta = b - mu*s_w/sigma.
  Per-core partial G^T[768,512] products are combined with a
  ReduceScatter laid out as [8,768,64] so core c receives exactly its
  own batch-slice [768,64] - batch-parallel phase 2 follows with no
  core-id-dependent code.

  Phase 2 (batch-parallel, 64 rows/core): PrimaryCaps (3x8 linears),
  squash, and 3 dynamic-routing iterations computed without
  materializing u_hat[B,192,2,64]:
     s_c   = (W2 * c)^T @ u           (contraction over (r,i)=1536)
     P_c   = W3_c^T @ v_c             (back-projected v)
     a_rc  = sum_b sum_i u * P_c      (agreement, fused mul+reduce)
  The batch-mean of the agreement is an AllReduce of [2,192] per
  routing iteration (2 of them).
"""

import os
import sys

import numpy as np

if "/opt/trn_rl_repo" not in sys.path:
    sys.path.insert(0, "/opt/trn_rl_repo")

import concourse.bass as bass  # noqa: E402
import concourse.mybir as mybir  # noqa: E402
import concourse.tile as tile  # noqa: E402
from concourse import bacc  # noqa: E402
from concourse.bass_utils import run_bass_kernel_spmd  # noqa: E402
from concourse.masks import make_identity  # noqa: E402

try:
    import ml_dtypes  # noqa: E402

    _BF16 = ml_dtypes.bfloat16
except Exception:  # pragma: no cover
    _BF16 = None

N_CORES = 8
B, BC = 512, 64  # batch, per-core batch
K, KC = 102400, 12800  # contraction dim, per-core slice
E = 768  # embedding
ET = E // 128  # e chunks
KT = KC // 128  # k tiles per core (100)
GROUP = 4  # k tiles per DMA
RI = 1536  # (route, in_cap) flat = 192*8
RT = RI // 128  # 12 tiles
NCLS = 2
OC = 64  # out caps channels
F32 = mybir.dt.float32

# phase-1 device dtype: "bf16" or "f32"
PHASE1_DT = os.environ.get("CAPS_P1_DT", "bf16")
# bisection: 1=phase1 only, 2=+RS, 3=+prim/squash/u2, 4=+routing it0+AR, 5=full
STOP = int(os.environ.get("CAPS_STOP", "5"))

_CACHE = {}


def _emit(nc, tc, const, loads, work, ps1, dram, io):
    dt1 = mybir.dt.bfloat16 if PHASE1_DT == "bf16" else F32
    amask = 0x7FFF if PHASE1_DT == "bf16" else 0x7FFFFFFF
    udt = mybir.dt.uint16 if PHASE1_DT == "bf16" else mybir.dt.uint32
    rg = [list(range(N_CORES))]
    dct_t, wp, beta, img_t, capt_t, wm2, bias3, w2, w3, y = io

    def debug_out(fill=None):
        out_sb = work.tile([BC, 128], F32, tag="outsb", name="outsb")
        nc.vector.memset(out_sb[:], 0.0)
        if fill is not None:
            fill(out_sb)
        nc.sync.dma_start(y[:], out_sb[:])

    eps_ln = const.tile([128, 1], F32)
    nc.vector.memset(eps_ln[:], 1e-12)
    eps_sq = const.tile([128, 1], F32)
    nc.vector.memset(eps_sq[:], 1e-7)

    # ---------------- phase 1: big matmul ----------------
    g_ps = [ps1.tile([128, B], F32, tag=f"g{ec}", name=f"g{ec}") for ec in range(ET)]
    n_loads = KT // GROUP
    for li in range(n_loads):
        k0 = li * GROUP * 128
        dct_tile = loads.tile([128, GROUP, B], dt1, tag="dct")
        nc.sync.dma_start(
            dct_tile[:],
            dct_t[k0 : k0 + GROUP * 128, :].rearrange("(s p) b -> p s b", p=128),
        )
        w_tile = loads.tile([128, GROUP, E], dt1, tag="w")
        nc.sync.dma_start(
            w_tile[:],
            wp[k0 : k0 + GROUP * 128, :].rearrange("(s p) e -> p s e", p=128),
        )
        dlog = loads.tile([128, GROUP, B], dt1, tag="dlog")
        nc.vector.tensor_scalar(
            dlog[:].bitcast(udt),
            dct_tile[:].bitcast(udt),
            amask,
            None,
            op0=mybir.AluOpType.bitwise_and,
        )
        nc.scalar.activation(
            dlog[:], dlog[:], mybir.ActivationFunctionType.Ln, bias=eps_ln[:]
        )
        for s in range(GROUP):
            kt = li * GROUP + s
            for ec in range(ET):
                nc.tensor.matmul(
                    g_ps[ec][:],
                    w_tile[:, s, ec * 128 : (ec + 1) * 128],
                    dlog[:, s, :],
                    start=(kt == 0),
                    stop=(kt == KT - 1),
                )

    # evacuate PSUM -> SBUF -> cc_in (laid out for ReduceScatter)
    cc_in = dram.tile([N_CORES, E, BC], F32)
    for ec in range(ET):
        g_sb = work.tile([128, B], F32, tag="gsb")
        nc.vector.tensor_copy(g_sb[:], g_ps[ec][:])
        nc.sync.dma_start(
            cc_in[:, ec * 128 : (ec + 1) * 128, :].rearrange("c p b -> p c b"),
            g_sb[:].rearrange("p (c b) -> p c b", c=N_CORES),
        )
    if STOP == 1:
        debug_out()
        return

    rs_out = dram.tile([E, BC], F32)
    nc.gpsimd.collective_compute(
        "ReduceScatter",
        mybir.AluOpType.add,
        replica_groups=rg,
        ins=[cc_in.opt()],
        outs=[rs_out.opt()],
    )
    if STOP == 2:
        debug_out(lambda o: nc.sync.dma_start(o[:, :OC], rs_out[:BC, :]))
        return

    # ---------------- phase 2 constants ----------------
    identity = const.tile([128, 128], F32)
    make_identity(nc, identity[:])
    beta_sb = const.tile([128, ET], F32)
    nc.sync.dma_start(beta_sb[:], beta[:].rearrange("(t p) -> p t", p=128))
    emb_sb = {}  # (m, et) -> [128, BC] tile (lhsT for prim matmuls)
    for m, src in ((0, img_t), (1, capt_t)):
        for et in range(ET):
            t = const.tile([128, BC], F32, tag=f"emb{m}_{et}", name=f"emb{m}_{et}")
            nc.sync.dma_start(t[:], src[et * 128 : (et + 1) * 128, :])
            emb_sb[(m, et)] = t
    wm2_sb = {}
    for m in range(3):
        for et in range(ET):
            t = const.tile([128, 512], F32, tag=f"wm2_{m}_{et}", name=f"wm2_{m}_{et}")
            nc.sync.dma_start(t[:], wm2[m, et * 128 : (et + 1) * 128, :])
            wm2_sb[(m, et)] = t
    bias_sb = []
    for m in range(3):
        t = const.tile([1, 512], F32, tag=f"bias{m}", name=f"bias{m}")
        nc.sync.dma_start(t[:], bias3[m : m + 1, :])
        bias_sb.append(t)
    ones1 = const.tile([1, BC], F32)
    nc.vector.memset(ones1[:], 1.0)
    w2_sb = []
    for t_ in range(RT):
        t = const.tile([128, 128], F32, tag=f"w2_{t_}", name=f"w2_{t_}")
        nc.sync.dma_start(t[:], w2[t_ * 128 : (t_ + 1) * 128, :])
        w2_sb.append(t)
    w3_sb = []
    for c in range(NCLS):
        t = const.tile([OC, RI], F32, tag=f"w3_{c}", name=f"w3_{c}")
        nc.sync.dma_start(t[:], w3[c])
        w3_sb.append(t)

    # ---------------- phase 2: embeddings + primary caps ----------------
    for et in range(ET):
        gp = work.tile([128, BC], F32, tag="gp")
        nc.sync.dma_start(gp[:], rs_out[et * 128 : (et + 1) * 128, :])
        t = const.tile([128, BC], F32, tag=f"emb2_{et}", name=f"emb2_{et}")
        nc.scalar.activation(
            t[:],
            gp[:],
            mybir.ActivationFunctionType.Relu,
            bias=beta_sb[:, et : et + 1],
        )
        emb_sb[(2, et)] = t

    upre = const.tile([BC, RI], F32, tag="upre")
    for m in range(3):
        pm = ps1.tile([BC, 512], F32, tag="pp", bufs=2, name="pm")
        for et in range(ET):
            nc.tensor.matmul(
                pm[:],
                emb_sb[(m, et)][:],
                wm2_sb[(m, et)][:],
                start=(et == 0),
                stop=False,
            )
        nc.tensor.matmul(pm[:], ones1[:], bias_sb[m][:], start=False, stop=True)
        nc.vector.tensor_copy(upre[:, m * 512 : (m + 1) * 512], pm[:])

    # squash over i (last 8): u = upre * f(sq),  sq = sum_i upre^2
    sq8 = work.tile([BC, RI], F32, tag="sq8", bufs=1)
    nc.vector.tensor_mul(sq8[:], upre[:], upre[:])
    usq = work.tile([BC, 192], F32, tag="usq")
    nc.vector.tensor_reduce(
        usq[:],
        sq8[:].rearrange("p (r i) -> p r i", i=8),
        axis=mybir.AxisListType.X,
        op=mybir.AluOpType.add,
    )
    t1 = work.tile([BC, 192], F32, tag="fa")
    nc.scalar.activation(
        t1[:], usq[:], mybir.ActivationFunctionType.Sqrt, bias=eps_sq[:BC, :]
    )
    t2 = work.tile([BC, 192], F32, tag="fb")
    nc.vector.tensor_scalar_add(t2[:], usq[:], 1.0)
    t3 = work.tile([BC, 192], F32, tag="fc")
    nc.vector.tensor_mul(t3[:], t1[:], t2[:])
    t4 = work.tile([BC, 192], F32, tag="fd")
    nc.vector.reciprocal(t4[:], t3[:])
    t5 = work.tile([BC, 192], F32, tag="fe")
    nc.vector.tensor_mul(t5[:], t4[:], usq[:])
    usquash = work.tile([BC, RI], F32, tag="usquash", bufs=1)
    nc.vector.tensor_tensor(
        usquash[:].rearrange("p (r i) -> p r i", i=8),
        upre[:].rearrange("p (r i) -> p r i", i=8),
        t5[:].broadcast_to([BC, 192, 8]),
        op=mybir.AluOpType.mult,
    )

    # transpose u -> u2[t] [(r,i)-tile, b]
    u2 = []
    for t_ in range(RT):
        tp = ps1.tile([128, BC], F32, tag="pp", bufs=2, name="tp")
        nc.tensor.transpose(
            tp[:], usquash[:, t_ * 128 : (t_ + 1) * 128], identity[:BC, :BC]
        )
        t = const.tile([128, BC], F32, tag=f"u2_{t_}", name=f"u2_{t_}")
        nc.vector.tensor_copy(t[:], tp[:])
        u2.append(t)
    if STOP == 3:
        debug_out(lambda o: nc.vector.tensor_copy(o[:, :OC], u2[0][:BC, :]))
        return

    # ---------------- dynamic routing ----------------
    ar_in = [dram.tile([NCLS, 192], F32, name=f"ar_in{i}") for i in range(2)]
    ar_out = [dram.tile([NCLS, 192], F32, name=f"ar_out{i}") for i in range(2)]
    c_dram = [dram.tile([192, NCLS], F32, name=f"c_dram{i}") for i in range(2)]
    b_cur = None  # [2,192] logits tile

    def digit_squash(s_sb, it, c):
        """elementwise v = sq*s/((1+sq)*sqrt(sq+eps)) on [OC, BC]."""
        sq = work.tile([OC, BC], F32, tag="dsq", name="dsq")
        nc.vector.tensor_mul(sq[:], s_sb[:], s_sb[:])
        d1 = work.tile([OC, BC], F32, tag="dd1", name="dd1")
        nc.scalar.activation(
            d1[:], sq[:], mybir.ActivationFunctionType.Sqrt, bias=eps_sq[:OC, :]
        )
        d2 = work.tile([OC, BC], F32, tag="dd2", name="dd2")
        nc.vector.tensor_scalar_add(d2[:], sq[:], 1.0)
        d3 = work.tile([OC, BC], F32, tag="dd3", name="dd3")
        nc.vector.tensor_mul(d3[:], d1[:], d2[:])
        d4 = work.tile([OC, BC], F32, tag="dd4", name="dd4")
        nc.vector.reciprocal(d4[:], d3[:])
        d5 = work.tile([OC, BC], F32, tag="dd5", name="dd5")
        nc.vector.tensor_mul(d5[:], d4[:], sq[:])
        v = work.tile([OC, BC], F32, tag=f"v{c}", name=f"v{c}")
        nc.vector.tensor_mul(v[:], d5[:], s_sb[:])
        return v

    vs = []
    for it in range(3):
        rnd = it - 1  # collective-round index for it>=1
        if it == 0:
            mset = w2_sb  # uniform c folded into evac scale 1/192
        else:
            # softmax(b_cur) over routes -> c_sm [2,192]
            mx = work.tile([NCLS, 1], F32, tag="smx", name="smx")
            nc.vector.tensor_reduce(
                mx[:], b_cur[:], axis=mybir.AxisListType.X, op=mybir.AluOpType.max
            )
            mxn = work.tile([NCLS, 1], F32, tag="smxn", name="smxn")
            nc.vector.tensor_scalar_mul(mxn[:], mx[:], -1.0)
            ex = work.tile([NCLS, 192], F32, tag="sex", name="sex")
            nc.scalar.activation(
                ex[:], b_cur[:], mybir.ActivationFunctionType.Exp, bias=mxn[:]
            )
            sm = work.tile([NCLS, 1], F32, tag="ssm", name="ssm")
            nc.vector.tensor_reduce(
                sm[:], ex[:], axis=mybir.AxisListType.X, op=mybir.AluOpType.add
            )
            rcp = work.tile([NCLS, 1], F32, tag="srcp", name="srcp")
            nc.vector.reciprocal(rcp[:], sm[:])
            c_sm = work.tile([NCLS, 192], F32, tag="scs", name="scs")
            nc.vector.tensor_scalar(
                c_sm[:], ex[:], rcp[:], None, op0=mybir.AluOpType.mult
            )
            # c [2,192] -> DRAM [192,2] -> broadcast-read into
            # c_exp [128, RT, 2]  (value c[16t+p//8, cls])
            nc.sync.dma_start(c_dram[rnd][:].rearrange("r c -> c r"), c_sm[:])
            c_exp = work.tile([128, RT, NCLS], F32, tag="cexp", name="cexp")
            for t_ in range(RT):
                nc.sync.dma_start(
                    c_exp[:, t_, :],
                    c_dram[rnd][16 * t_ : 16 * (t_ + 1), :]
                    .broadcast_to([16, NCLS, 8])
                    .rearrange("j c r -> j r c"),
                )
            mset = []
            for t_ in range(RT):
                msc = work.tile([128, 128], F32, tag="msc", bufs=3, name="msc")
                for c in range(NCLS):
                    nc.vector.tensor_scalar(
                        msc[:, c * OC : (c + 1) * OC],
                        w2_sb[t_][:, c * OC : (c + 1) * OC],
                        c_exp[:, t_, c : c + 1],
                        None,
                        op0=mybir.AluOpType.mult,
                    )
                mset.append(msc)

        # s_c = mset^T @ u2 (per class), then squash -> v_c
        vs = []
        for c in range(NCLS):
            s_ps = ps1.tile([OC, BC], F32, tag="pp", bufs=2, name="s_ps")
            for t_ in range(RT):
                nc.tensor.matmul(
                    s_ps[:],
                    mset[t_][:, c * OC : (c + 1) * OC],
                    u2[t_][:],
                    start=(t_ == 0),
                    stop=(t_ == RT - 1),
                )
            s_sb = work.tile([OC, BC], F32, tag=f"ssb{c}", name=f"ssb{c}")
            nc.scalar.mul(s_sb[:], s_ps[:], (1.0 / 192.0) if it == 0 else 1.0)
            if it == 0 and c == 0 and STOP == 31:
                debug_out(lambda o: nc.vector.tensor_copy(o[:, :OC], s_sb[:]))
                return
            vs.append(digit_squash(s_sb, it, c))
        if it == 0 and STOP == 32:
            debug_out(lambda o: nc.vector.tensor_copy(o[:, :OC], vs[0][:]))
            return

        if it < 2:
            # agreement: abar[r,c] = sum_b sum_i u2 * (W3_c^T @ v_c)
            for c in range(NCLS):
                dcat = work.tile([128, RT], F32, tag=f"dcat{c}", name=f"dcat{c}")
                for t_ in range(RT):
                    pc = ps1.tile([128, BC], F32, tag="pp", bufs=2, name="pc")
                    nc.tensor.matmul(
                        pc[:],
                        w3_sb[c][:, t_ * 128 : (t_ + 1) * 128],
                        vs[c][:],
                        start=True,
                        stop=True,
                    )
                    if it == 0 and c == 0 and t_ == 0 and STOP == 331:
                        debug_out(lambda o: nc.vector.tensor_copy(o[:, :OC], pc[:BC, :]))
                        return
                    prod = work.tile([128, BC], F32, tag="prod", name="prod")
                    nc.vector.tensor_mul(prod[:], u2[t_][:], pc[:])
                    nc.vector.tensor_reduce(
                        dcat[:, t_ : t_ + 1],
                        prod[:],
                        axis=mybir.AxisListType.X,
                        op=mybir.AluOpType.add,
                    )
                if it == 0 and c == 0 and STOP == 332:
                    debug_out(lambda o: nc.vector.tensor_copy(o[:, :RT], dcat[:BC, :]))
                    return
                dtp = ps1.tile([RT, 128], F32, tag="pp", bufs=2, name="dtp")
                nc.tensor.transpose(dtp[:], dcat[:], identity[:])
                if it == 0 and c == 0 and STOP == 333:
                    debug_out(lambda o: nc.vector.tensor_copy(o[:RT, :], dtp[:]))
                    return
                abar = work.tile([RT, 16], F32, tag=f"abar{c}", name=f"abar{c}")
                nc.vector.tensor_reduce(
                    abar[:],
                    dtp[:].rearrange("p (r i) -> p r i", i=8),
                    axis=mybir.AxisListType.X,
                    op=mybir.AluOpType.add,
                )
                nc.sync.dma_start(
                    ar_in[it][c, :].rearrange("(t j) -> t j", t=RT), abar[:]
                )
            if it == 0 and STOP == 33:
                debug_out(lambda o: nc.vector.tensor_copy(o[:RT, :16], abar[:]))
                return
            nc.gpsimd.collective_compute(
                "AllReduce",
                mybir.AluOpType.add,
                replica_groups=rg,
                ins=[ar_in[it].opt()],
                outs=[ar_out[it].opt()],
            )
            if it == 0 and STOP == 34:
                debug_out()
                return
            ld = work.tile([NCLS, 192], F32, tag=f"arld{it}", name=f"arld{it}")
            nc.sync.dma_start(ld[:], ar_out[it][:])
            b_new = work.tile([NCLS, 192], F32, tag=f"bcur{it}", name=f"bcur{it}")
            if it == 0:
                nc.scalar.mul(b_new[:], ld[:], 1.0 / B)
            else:
                scaled = work.tile([NCLS, 192], F32, tag="arsc", name="arsc")
                nc.vector.tensor_scalar_mul(scaled[:], ld[:], 1.0 / B)
                nc.vector.tensor_add(b_new[:], b_cur[:], scaled[:])
            b_cur = b_new
        if it == 0 and STOP == 4:
            debug_out(lambda o: nc.vector.tensor_copy(o[:, :OC], vs[0][:]))
            return

    # final output: y[b, (c,o)] via PE transpose of v_c
    out_sb = work.tile([BC, 128], F32, tag="outsb", name="outsb")
    for c in range(NCLS):
        vt = ps1.tile([BC, OC], F32, tag="pp", bufs=2, name="vt")
        nc.tensor.transpose(vt[:], vs[c][:], identity[:OC, :OC])
        nc.vector.tensor_copy(out_sb[:, c * OC : (c + 1) * OC], vt[:])
    nc.sync.dma_start(y[:], out_sb[:])


def _build_program():
    dt1 = mybir.dt.bfloat16 if PHASE1_DT == "bf16" else F32
    nc = bacc.Bacc(num_devices=N_CORES)

    dct_t = nc.declare_dram_parameter("dct_t", [KC, B], dt1, isOutput=False)
    wp = nc.declare_dram_parameter("wp", [KC, E], dt1, isOutput=False)
    beta = nc.declare_dram_parameter("beta", [E], F32, isOutput=False)
    img_t = nc.declare_dram_parameter("img_t", [E, BC], F32, isOutput=False)
    capt_t = nc.declare_dram_parameter("capt_t", [E, BC], F32, isOutput=False)
    wm2 = nc.declare_dram_parameter("wm2", [3, E, 512], F32, isOutput=False)
    bias3 = nc.declare_dram_parameter("bias3", [3, 512], F32, isOutput=False)
    w2 = nc.declare_dram_parameter("w2", [RI, 128], F32, isOutput=False)
    w3 = nc.declare_dram_parameter("w3", [NCLS, OC, RI], F32, isOutput=False)
    y = nc.declare_dram_parameter("y", [BC, 128], F32, isOutput=True)
    io = (dct_t, wp, beta, img_t, capt_t, wm2, bias3, w2, w3, y)

    with tile.TileContext(nc) as tc:
        with (
            tc.tile_pool(name="const", bufs=1) as const,
            tc.tile_pool(name="loads", bufs=3 if PHASE1_DT == "bf16" else 2) as loads,
            tc.tile_pool(name="work", bufs=2) as work,
            tc.tile_pool(name="ps1", bufs=1, space="PSUM") as ps1,
            tc.tile_pool(name="dram", bufs=1, space="DRAM") as dram,
        ):
            _emit(nc, tc, const, loads, work, ps1, dram, io)

    nc.compile()
    return nc


def _host_prep(inputs):
    """Numpy-side sharding/layout prep. Returns per-core input maps."""
    img_emb = np.asarray(inputs["img_emb"], dtype=np.float32)
    capt_emb = np.asarray(inputs["capt_emb"], dtype=np.float32)
    dct = np.asarray(inputs["DCT_features"], dtype=np.float32).reshape(B, K)
    w_emb = np.asarray(inputs["W_emb"], dtype=np.float32)
    b_emb = np.asarray(inputs["b_emb"], dtype=np.float32)
    w_digit = np.asarray(inputs["W_digit"], dtype=np.float32)

    dlog = np.log(np.abs(dct) + 1e-12)
    mu = float(dlog.mean(dtype=np.float64))
    sigma = float(dlog.std(ddof=1, dtype=np.float64))
    s_w = w_emb.sum(axis=1, dtype=np.float64)
    beta = (b_emb - (mu / sigma) * s_w).astype(np.float32)

    np_dt1 = _BF16 if PHASE1_DT == "bf16" else np.float32
    dct_T = np.ascontiguousarray(dct.T).astype(np_dt1)  # [K, B]
    wp = np.ascontiguousarray(w_emb.T / sigma).astype(np_dt1)  # [K, E]

    wm2 = np.stack(
        [
            np.ascontiguousarray(
                np.asarray(inputs[f"W_{m}"], dtype=np.float32).transpose(2, 1, 0)
            ).reshape(E, 512)
            for m in ("img", "capt", "dct")
        ]
    )  # [3, E, 512]
    bias3 = np.stack(
        [
            np.ascontiguousarray(
                np.asarray(inputs[f"b_{m}"], dtype=np.float32).T
            ).reshape(512)
            for m in ("img", "capt", "dct")
        ]
    )  # [3, 512]
    w2 = np.ascontiguousarray(w_digit.transpose(0, 3, 1, 2)).reshape(RI, 128)
    w3 = np.stack(
        [
            np.ascontiguousarray(w_digit[:, c].transpose(1, 0, 2)).reshape(OC, RI)
            for c in range(NCLS)
        ]
    )  # [2, OC, RI]

    in_maps = []
    for c in range(N_CORES):
        in_maps.append(
            {
                "dct_t": np.ascontiguousarray(dct_T[c * KC : (c + 1) * KC]),
                "wp": np.ascontiguousarray(wp[c * KC : (c + 1) * KC]),
                "beta": beta,
                "img_t": np.ascontiguousarray(img_emb[c * BC : (c + 1) * BC].T),
                "capt_t": np.ascontiguousarray(capt_emb[c * BC : (c + 1) * BC].T),
                "wm2": wm2,
                "bias3": bias3,
                "w2": w2,
                "w3": w3,
            }
        )
    return in_maps


def kernel(**inputs) -> np.ndarray:
    if "nc" not in _CACHE:
        _CACHE["nc"] = _build_program()
    nc = _CACHE["nc"]
    in_maps = _host_prep(inputs)
    trace = bool(int(os.environ.get("CAPS_TRACE", "0")))
    res = run_bass_kernel_spmd(nc, in_maps, list(range(N_CORES)), trace=trace)
    _CACHE["last_result"] = res
    out = np.concatenate(
        [res.results[c]["y"].reshape(BC, NCLS, OC) for c in range(N_CORES)], axis=0
    )
    return out[:, :, :, None]
